# revision 27
# baseline (speedup 1.0000x reference)
"""Trainium2 Bass kernel for nn_AttnHead_81028853006993.

LayerNorm + affine + fused QKV + 4-head attention with gathered relative-position
mask + output projection, for x:[8, 2048, 512] f32.

Sharding: data-parallel over batch — 8 batches onto 8 NeuronCores, no collectives.
Each core runs the full per-batch attention head stack. All matmuls in bf16
(f32 matmuls cost two PE passes on TRN2); statistics/softmax accumulation in f32.
"""

import os
import sys

import numpy as np

for _p in ("/opt/trn_rl_repo",):
    if _p not in sys.path:
        sys.path.insert(0, _p)

import ml_dtypes  # noqa: E402

B, T, N = 8, 2048, 512
H, HD = 4, 128
P = 128
NT = T // P  # 16 token tiles
KC = N // P  # 4 embed chunks
FQK = 2 * N // P  # 8 feature chunks for fused QK
EPS = 1e-5
NEG = -1e9

LAST_RESULTS = None
_CACHE = {}


def _build_nc():
    import concourse.bacc as bacc
    import concourse.mybir as mybir
    import concourse.tile as tile
    from concourse.bass import AP, ts
    from concourse.masks import make_identity

    f32 = mybir.dt.float32
    bf16 = mybir.dt.bfloat16
    FI = mybir.ActivationFunctionType

    nc = bacc.Bacc("TRN2", target_bir_lowering=False, debug=False, num_devices=8)

    x_d = nc.dram_tensor("x", [T, N], f32, kind="ExternalInput")
    mask_d = nc.dram_tensor("maskadd", [T, T], bf16, kind="ExternalInput")
    wqk_d = nc.dram_tensor("wqk", [N, 2 * N], bf16, kind="ExternalInput")
    wv_d = nc.dram_tensor("wv", [N, N], bf16, kind="ExternalInput")
    wp_d = nc.dram_tensor("wproj", [N, N], bf16, kind="ExternalInput")
    bqk_d = nc.dram_tensor("bqk", [2 * N], f32, kind="ExternalInput")
    ob_d = nc.dram_tensor("obias", [N], f32, kind="ExternalInput")
    out_d = nc.dram_tensor("out", [T, N], f32, kind="ExternalOutput")

    with tile.TileContext(nc) as tc:
        from contextlib import ExitStack

        with ExitStack() as ctx:
            singles = ctx.enter_context(tc.tile_pool(name="singles", bufs=1))
            big = ctx.enter_context(tc.tile_pool(name="big", bufs=1))
            xtp = ctx.enter_context(tc.tile_pool(name="xtp", bufs=2))
            lnx = ctx.enter_context(tc.tile_pool(name="lnx", bufs=3))
            smallp = ctx.enter_context(tc.tile_pool(name="smallp", bufs=8))
            maskp = ctx.enter_context(tc.tile_pool(name="maskp", bufs=3))
            probsp = ctx.enter_context(tc.tile_pool(name="probsp", bufs=3))
            ptp = ctx.enter_context(tc.tile_pool(name="ptp", bufs=3))
            attnp = ctx.enter_context(tc.tile_pool(name="attnp", bufs=2))
            attntp = ctx.enter_context(tc.tile_pool(name="attntp", bufs=2))
            outp = ctx.enter_context(tc.tile_pool(name="outp", bufs=3))
            # PSUM: 2x [128,1024]f32 (4) + 2x [128,512] (2) + 2x [128,512] (2)
            ps_bigp = ctx.enter_context(
                tc.tile_pool(name="ps_bigp", bufs=2, space="PSUM")
            )
            ps_tp = ctx.enter_context(tc.tile_pool(name="ps_tp", bufs=2, space="PSUM"))
            ps_op = ctx.enter_context(tc.tile_pool(name="ps_op", bufs=2, space="PSUM"))

            # ---- constants / weights ----
            ident_b = singles.tile([P, P], bf16)
            make_identity(nc, ident_b)
            eps_t = singles.tile([P, 1], f32)
            nc.vector.memset(eps_t, EPS)

            wqk_sb = singles.tile([P, KC, 2 * N], bf16)
            nc.sync.dma_start(
                out=wqk_sb, in_=wqk_d.ap().rearrange("(kc p) f -> p kc f", p=P)
            )
            wv_sb = singles.tile([P, KC, N], bf16)
            nc.sync.dma_start(
                out=wv_sb, in_=wv_d.ap().rearrange("(kc p) f -> p kc f", p=P)
            )
            wp_sb = singles.tile([P, KC, N], bf16)
            nc.sync.dma_start(
                out=wp_sb, in_=wp_d.ap().rearrange("(kc p) f -> p kc f", p=P)
            )
            bqk_sb = singles.tile([P, FQK], f32)
            nc.sync.dma_start(
                out=bqk_sb, in_=bqk_d.ap().rearrange("(fc p) -> p fc", p=P)
            )
            # obias broadcast to all 128 partitions (DMA partition-stride 0)
            ob_bc = singles.tile([P, N], f32)
            _o = ob_d.ap()
            nc.sync.dma_start(
                out=ob_bc,
                in_=AP(tensor=_o.tensor, offset=_o.offset, ap=[[0, P]] + list(_o.ap)),
            )

            qkT = big.tile([P, FQK, T], bf16)  # Q^T,K^T feature-major
            vsb = big.tile([P, NT, N], bf16)  # V token-major

            def alt_copy(idx, out, in_):
                if idx % 2 == 0:
                    nc.vector.tensor_copy(out=out, in_=in_)
                else:
                    nc.scalar.copy(out=out, in_=in_)

            # ---- fused LN + QKV phase, per 512-token chunk ----
            copy_flip = 0
            for tj in range(4):
                xtc = xtp.tile([P, KC, 4 * P], bf16)  # x-hat^T for this token chunk
                for s in range(4):
                    i = tj * 4 + s
                    x_tile = lnx.tile([P, N], f32)
                    nc.sync.dma_start(out=x_tile, in_=x_d.ap()[ts(i, P), :])
                    stats = smallp.tile([P, 6], f32)
                    nc.vector.bn_stats(out=stats, in_=x_tile)
                    mv = smallp.tile([P, 2], f32)
                    nc.vector.bn_aggr(out=mv, in_=stats)
                    sig = smallp.tile([P, 1], f32)
                    nc.scalar.activation(
                        out=sig, in_=mv[:, 1:2], func=FI.Sqrt, bias=eps_t
                    )
                    rstd = smallp.tile([P, 1], f32)
                    nc.vector.reciprocal(out=rstd, in_=sig)
                    # x-hat = (x - mean) * rstd, cast to bf16
                    xh = lnx.tile([P, N], bf16)
                    nc.vector.tensor_scalar(
                        out=xh,
                        in0=x_tile,
                        scalar1=mv[:, 0:1],
                        scalar2=rstd,
                        op0=mybir.AluOpType.subtract,
                        op1=mybir.AluOpType.mult,
                    )
                    ps_x = ps_tp.tile([P, 4 * P], bf16, tag="pst")
                    for kc in range(KC):
                        nc.tensor.matmul(
                            ps_x[:, ts(kc, P)],
                            xh[:, ts(kc, P)],
                            ident_b,
                            start=(kc == 0),
                            stop=(kc == KC - 1),
                            is_transpose=True,
                        )
                    alt_copy(
                        copy_flip,
                        xtc[:, :, ts(s, P)],
                        ps_x.rearrange("p (kc q) -> p kc q", kc=KC),
                    )
                    copy_flip += 1
                # QK^T for this token chunk: out[feat, tok]
                for g in range(4):
                    ps = ps_bigp.tile([P, 1024], f32, tag="psb")
                    for half in range(2):
                        fc = g * 2 + half
                        for kc in range(KC):
                            nc.tensor.matmul(
                                ps[:, ts(half, 512)],
                                wqk_sb[:, kc, ts(fc, P)],
                                xtc[:, kc, :],
                                start=(kc == 0),
                                stop=(kc == KC - 1),
                            )
                    for half in range(2):
                        fc = g * 2 + half
                        if copy_flip % 2 == 0:
                            nc.vector.tensor_scalar_add(
                                out=qkT[:, fc, ts(tj, 512)],
                                in0=ps[:, ts(half, 512)],
                                scalar1=bqk_sb[:, fc : fc + 1],
                            )
                        else:
                            nc.scalar.activation(
                                out=qkT[:, fc, ts(tj, 512)],
                                in_=ps[:, ts(half, 512)],
                                func=FI.Identity,
                                bias=bqk_sb[:, fc : fc + 1],
                            )
                        copy_flip += 1
                # V for this token chunk: out[tok, feat] -> bf16
                for s in range(4):
                    tm = tj * 4 + s
                    ps2 = ps_op.tile([P, N], f32, tag="pso")
                    for kc in range(KC):
                        nc.tensor.matmul(
                            ps2,
                            xtc[:, kc, ts(s, P)],
                            wv_sb[:, kc, :],
                            start=(kc == 0),
                            stop=(kc == KC - 1),
                        )
                    alt_copy(copy_flip, vsb[:, tm, :], ps2)
                    copy_flip += 1

            # ---- attention phase, per 128-query tile ----
            for i in range(NT):
                mask_t = maskp.tile([P, T], bf16, tag="mask")
                nc.sync.dma_start(out=mask_t, in_=mask_d.ap()[ts(i, P), :])
                ps_pv = ps_op.tile([P, N], f32, tag="pso")
                attn_sb = attnp.tile([P, N], bf16)
                for h in range(H):
                    probs = probsp.tile([P, T], bf16)
                    den = smallp.tile([P, 2], f32)
                    for hf in range(2):
                        ps_s = ps_bigp.tile([P, 1024], f32, tag="psb")
                        for j2 in range(2):
                            j = hf * 2 + j2
                            nc.tensor.matmul(
                                ps_s[:, ts(j2, 512)],
                                ident_b,
                                mask_t[:, ts(j, 512)],
                                start=True,
                                stop=False,
                            )
                        for j2 in range(2):
                            j = hf * 2 + j2
                            nc.tensor.matmul(
                                ps_s[:, ts(j2, 512)],
                                qkT[:, h, ts(i, P)],
                                qkT[:, H + h, ts(j, 512)],
                                start=False,
                                stop=True,
                            )
                        nc.scalar.activation(
                            out=probs[:, ts(hf, 1024)],
                            in_=ps_s,
                            func=FI.Exp,
                            accum_out=den[:, hf : hf + 1],
                        )
                    denom = smallp.tile([P, 1], f32)
                    nc.vector.tensor_tensor(
                        out=denom,
                        in0=den[:, 0:1],
                        in1=den[:, 1:2],
                        op=mybir.AluOpType.add,
                    )
                    recip = smallp.tile([P, 1], f32)
                    nc.vector.reciprocal(out=recip, in_=denom)
                    probsT = ptp.tile([P, NT, P], bf16)
                    for c in range(4):
                        ps_t = ps_tp.tile([P, 512], bf16, tag="pst")
                        for k in range(4):
                            nc.tensor.matmul(
                                ps_t[:, ts(k, P)],
                                probs[:, ts(c * 4 + k, P)],
                                ident_b,
                                start=(k == 0),
                                stop=(k == 3),
                                is_transpose=True,
                            )
                        nc.vector.tensor_copy(
                            out=probsT[:, c * 4 : (c + 1) * 4, :],
                            in_=ps_t.rearrange("p (c q) -> p c q", c=4),
                        )
                    for m in range(NT):
                        nc.tensor.matmul(
                            ps_pv[:, ts(h, HD)],
                            probsT[:, m, :],
                            vsb[:, m, ts(h, HD)],
                            start=(m == 0),
                            stop=(m == NT - 1),
                        )
                    nc.vector.tensor_scalar_mul(
                        out=attn_sb[:, ts(h, HD)],
                        in0=ps_pv[:, ts(h, HD)],
                        scalar1=recip,
                    )
                # project: out[tok, feat] = attnT.T @ Wp + obias
                ps_at = ps_tp.tile([P, N], bf16, tag="pst")
                for k in range(KC):
                    nc.tensor.matmul(
                        ps_at[:, ts(k, P)],
                        attn_sb[:, ts(k, P)],
                        ident_b,
                        start=(k == 0),
                        stop=(k == KC - 1),
                        is_transpose=True,
                    )
                attnT = attntp.tile([P, KC, P], bf16)
                alt_copy(
                    copy_flip, attnT, ps_at.rearrange("p (kc q) -> p kc q", kc=KC)
                )
                copy_flip += 1
                ps_pr = ps_op.tile([P, N], f32, tag="pso")
                for c in range(KC):
                    nc.tensor.matmul(
                        ps_pr,
                        attnT[:, c, :],
                        wp_sb[:, c, :],
                        start=(c == 0),
                        stop=(c == KC - 1),
                    )
                out_sb = outp.tile([P, N], f32)
                nc.vector.tensor_tensor(
                    out=out_sb, in0=ps_pr, in1=ob_bc, op=mybir.AluOpType.add
                )
                nc.sync.dma_start(out=out_d.ap()[ts(i, P), :], in_=out_sb)

    nc.compile()
    return nc


def _get_nc():
    if "nc" not in _CACHE:
        _CACHE["nc"] = _build_nc()
    return _CACHE["nc"]


def _prep_host(x, pos_emb, ln_w, ln_b, aff_w, aff_b, W_qkv, mask_table, W_proj):
    f = np.float32
    bf = ml_dtypes.bfloat16
    x = np.asarray(x, f)
    pos_emb = np.asarray(pos_emb)
    ln_w = np.asarray(ln_w, f)
    ln_b = np.asarray(ln_b, f)
    aff_w = np.asarray(aff_w, f)
    aff_b = np.asarray(aff_b, f)
    W_qkv = np.asarray(W_qkv, f)
    mask_table = np.asarray(mask_table)
    W_proj = np.asarray(W_proj, f)

    s = ln_w * aff_w
    c = ln_b * aff_w + aff_b
    Wf = (s[:, None] * W_qkv).astype(f)
    bf_ = (c @ W_qkv).astype(f)
    scale = f(1.0 / np.sqrt(HD))
    Wqk = np.concatenate([Wf[:, :N] * scale, Wf[:, N : 2 * N]], axis=1)
    Wqk = np.ascontiguousarray(Wqk).astype(bf)
    bqk = np.concatenate([bf_[:N] * scale, bf_[N : 2 * N]]).astype(f)
    Wv = np.ascontiguousarray(Wf[:, 2 * N :]).astype(bf)
    bv = bf_[2 * N :]
    obias = (bv @ W_proj).astype(f)
    Wp = np.ascontiguousarray(W_proj).astype(bf)
    maskadd = np.where(mask_table[pos_emb], f(0.0), f(NEG)).astype(bf)
    return x, maskadd, Wqk, Wv, Wp, bqk, obias


def _install_ntff_hook():
    """Provide the antenv.axon_hooks shim missing from this image so
    run_bass_kernel_spmd(trace=True) can capture NTFF profiles."""
    import types

    try:
        from antenv.axon_hooks import get_axon_ntff_profile_hook  # noqa: F401

        return
    except ImportError:
        pass
    try:
        import antenv
        from trn_agent_boot.trn_boot import _ntff_profile_via_ctypes

        hook = _ntff_profile_via_ctypes("/opt/axon/libaxon_pjrt.so")
        mod = types.ModuleType("antenv.axon_hooks")
        _h = [hook]
        mod.set_axon_ntff_profile_hook = lambda h: _h.__setitem__(0, h)
        mod.get_axon_ntff_profile_hook = lambda: _h[0]
        sys.modules["antenv.axon_hooks"] = mod
        antenv.axon_hooks = mod
    except Exception as e:  # pragma: no cover
        print(f"ntff hook install failed: {e}")


def kernel(x, pos_emb, ln_w, ln_b, aff_w, aff_b, W_qkv, mask_table, W_proj):
    global LAST_RESULTS
    from concourse.bass_utils import run_bass_kernel_spmd

    x, maskadd, Wqk, Wv, Wp, bqk, obias = _prep_host(
        x, pos_emb, ln_w, ln_b, aff_w, aff_b, W_qkv, mask_table, W_proj
    )
    nc = _get_nc()
    in_maps = [
        {
            "x": np.ascontiguousarray(x[i]),
            "maskadd": maskadd,
            "wqk": Wqk,
            "wv": Wv,
            "wproj": Wp,
            "bqk": bqk,
            "obias": obias,
        }
        for i in range(B)
    ]
    trace = bool(int(os.environ.get("ATTN_TRACE", "0")))
    if trace:
        _install_ntff_hook()
    res = run_bass_kernel_spmd(
        nc, in_maps, core_ids=list(range(B)), trace=trace
    )
    LAST_RESULTS = res
    out = np.stack([np.asarray(r["out"]) for r in res.results], axis=0)
    return out.astype(np.float32)


# revision 28
# speedup vs baseline: 1.0108x; 1.0108x over previous
"""Trainium2 Bass kernel for nn_AttnHead_81028853006993.

LayerNorm + affine + fused QKV + 4-head attention with gathered relative-position
mask + output projection, for x:[8, 2048, 512] f32.

Sharding: data-parallel over batch — 8 batches onto 8 NeuronCores, no collectives.
Each core runs the full per-batch attention head stack. All matmuls in bf16
(f32 matmuls cost two PE passes on TRN2); statistics/softmax accumulation in f32.
"""

import os
import sys

import numpy as np

for _p in ("/opt/trn_rl_repo",):
    if _p not in sys.path:
        sys.path.insert(0, _p)

import ml_dtypes  # noqa: E402

B, T, N = 8, 2048, 512
H, HD = 4, 128
P = 128
NT = T // P  # 16 token tiles
KC = N // P  # 4 embed chunks
FQK = 2 * N // P  # 8 feature chunks for fused QK
EPS = 1e-5
NEG = -1e9

LAST_RESULTS = None
_CACHE = {}


def _build_nc():
    import concourse.bacc as bacc
    import concourse.mybir as mybir
    import concourse.tile as tile
    from concourse.bass import AP, ts
    from concourse.masks import make_identity

    f32 = mybir.dt.float32
    bf16 = mybir.dt.bfloat16
    FI = mybir.ActivationFunctionType

    nc = bacc.Bacc("TRN2", target_bir_lowering=False, debug=False, num_devices=8)

    x_d = nc.dram_tensor("x", [T, N], f32, kind="ExternalInput")
    mask_d = nc.dram_tensor("maskadd", [T, T], bf16, kind="ExternalInput")
    wqk_d = nc.dram_tensor("wqk", [N, 2 * N], bf16, kind="ExternalInput")
    wv_d = nc.dram_tensor("wv", [N, N], bf16, kind="ExternalInput")
    wp_d = nc.dram_tensor("wproj", [N, N], bf16, kind="ExternalInput")
    bqk_d = nc.dram_tensor("bqk", [2 * N], f32, kind="ExternalInput")
    ob_d = nc.dram_tensor("obias", [N], f32, kind="ExternalInput")
    out_d = nc.dram_tensor("out", [T, N], f32, kind="ExternalOutput")

    with tile.TileContext(nc) as tc:
        from contextlib import ExitStack

        with ExitStack() as ctx:
            singles = ctx.enter_context(tc.tile_pool(name="singles", bufs=1))
            big = ctx.enter_context(tc.tile_pool(name="big", bufs=1))
            xtp = ctx.enter_context(tc.tile_pool(name="xtp", bufs=2))
            lnx = ctx.enter_context(tc.tile_pool(name="lnx", bufs=3))
            smallp = ctx.enter_context(tc.tile_pool(name="smallp", bufs=8))
            maskp = ctx.enter_context(tc.tile_pool(name="maskp", bufs=3))
            probsp = ctx.enter_context(tc.tile_pool(name="probsp", bufs=3))
            ptp = ctx.enter_context(tc.tile_pool(name="ptp", bufs=3))
            attnp = ctx.enter_context(tc.tile_pool(name="attnp", bufs=2))
            attntp = ctx.enter_context(tc.tile_pool(name="attntp", bufs=2))
            outp = ctx.enter_context(tc.tile_pool(name="outp", bufs=3))
            # PSUM: 2x [128,1024]f32 (4) + 2x [128,512] (2) + 2x [128,512] (2)
            ps_bigp = ctx.enter_context(
                tc.tile_pool(name="ps_bigp", bufs=2, space="PSUM")
            )
            ps_tp = ctx.enter_context(tc.tile_pool(name="ps_tp", bufs=2, space="PSUM"))
            ps_op = ctx.enter_context(tc.tile_pool(name="ps_op", bufs=2, space="PSUM"))

            # ---- constants / weights ----
            ident_b = singles.tile([P, P], bf16)
            make_identity(nc, ident_b)
            eps_t = singles.tile([P, 1], f32)
            nc.vector.memset(eps_t, EPS)

            wqk_sb = singles.tile([P, KC, 2 * N], bf16)
            nc.sync.dma_start(
                out=wqk_sb, in_=wqk_d.ap().rearrange("(kc p) f -> p kc f", p=P)
            )
            wv_sb = singles.tile([P, KC, N], bf16)
            nc.sync.dma_start(
                out=wv_sb, in_=wv_d.ap().rearrange("(kc p) f -> p kc f", p=P)
            )
            wp_sb = singles.tile([P, KC, N], bf16)
            nc.sync.dma_start(
                out=wp_sb, in_=wp_d.ap().rearrange("(kc p) f -> p kc f", p=P)
            )
            bqk_sb = singles.tile([P, FQK], f32)
            nc.sync.dma_start(
                out=bqk_sb, in_=bqk_d.ap().rearrange("(fc p) -> p fc", p=P)
            )
            # obias broadcast to all 128 partitions (DMA partition-stride 0)
            ob_bc = singles.tile([P, N], f32)
            _o = ob_d.ap()
            nc.sync.dma_start(
                out=ob_bc,
                in_=AP(tensor=_o.tensor, offset=_o.offset, ap=[[0, P]] + list(_o.ap)),
            )

            qkT = big.tile([P, FQK, T], bf16)  # Q^T,K^T feature-major
            vsb = big.tile([P, NT, N], bf16)  # V token-major

            def alt_copy(idx, out, in_):
                if idx % 2 == 0:
                    nc.vector.tensor_copy(out=out, in_=in_)
                else:
                    nc.scalar.copy(out=out, in_=in_)

            # ---- fused LN + QKV phase, per 512-token chunk ----
            copy_flip = 0
            for tj in range(4):
                xtc = xtp.tile([P, KC, 4 * P], bf16)  # x-hat^T for this token chunk
                for s in range(4):
                    i = tj * 4 + s
                    x_tile = lnx.tile([P, N], f32)
                    nc.sync.dma_start(out=x_tile, in_=x_d.ap()[ts(i, P), :])
                    stats = smallp.tile([P, 6], f32)
                    nc.vector.bn_stats(out=stats, in_=x_tile)
                    mv = smallp.tile([P, 2], f32)
                    nc.vector.bn_aggr(out=mv, in_=stats)
                    sig = smallp.tile([P, 1], f32)
                    nc.scalar.activation(
                        out=sig, in_=mv[:, 1:2], func=FI.Sqrt, bias=eps_t
                    )
                    rstd = smallp.tile([P, 1], f32)
                    nc.vector.reciprocal(out=rstd, in_=sig)
                    # x-hat = (x - mean) * rstd, cast to bf16
                    xh = lnx.tile([P, N], bf16)
                    nc.vector.tensor_scalar(
                        out=xh,
                        in0=x_tile,
                        scalar1=mv[:, 0:1],
                        scalar2=rstd,
                        op0=mybir.AluOpType.subtract,
                        op1=mybir.AluOpType.mult,
                    )
                    ps_x = ps_tp.tile([P, 4 * P], bf16, tag="pst")
                    for kc in range(KC):
                        nc.tensor.matmul(
                            ps_x[:, ts(kc, P)],
                            xh[:, ts(kc, P)],
                            ident_b,
                            start=(kc == 0),
                            stop=(kc == KC - 1),
                            is_transpose=True,
                        )
                    alt_copy(
                        copy_flip,
                        xtc[:, :, ts(s, P)],
                        ps_x.rearrange("p (kc q) -> p kc q", kc=KC),
                    )
                    copy_flip += 1
                # QK^T for this token chunk: out[feat, tok]
                for g in range(4):
                    ps = ps_bigp.tile([P, 1024], f32, tag="psb")
                    for half in range(2):
                        fc = g * 2 + half
                        for kc in range(KC):
                            nc.tensor.matmul(
                                ps[:, ts(half, 512)],
                                wqk_sb[:, kc, ts(fc, P)],
                                xtc[:, kc, :],
                                start=(kc == 0),
                                stop=(kc == KC - 1),
                            )
                    for half in range(2):
                        fc = g * 2 + half
                        if copy_flip % 2 == 0:
                            nc.vector.tensor_scalar_add(
                                out=qkT[:, fc, ts(tj, 512)],
                                in0=ps[:, ts(half, 512)],
                                scalar1=bqk_sb[:, fc : fc + 1],
                            )
                        else:
                            nc.scalar.activation(
                                out=qkT[:, fc, ts(tj, 512)],
                                in_=ps[:, ts(half, 512)],
                                func=FI.Identity,
                                bias=bqk_sb[:, fc : fc + 1],
                            )
                        copy_flip += 1
                # V for this token chunk: out[tok, feat] -> bf16
                for s in range(4):
                    tm = tj * 4 + s
                    ps2 = ps_op.tile([P, N], f32, tag="pso")
                    for kc in range(KC):
                        nc.tensor.matmul(
                            ps2,
                            xtc[:, kc, ts(s, P)],
                            wv_sb[:, kc, :],
                            start=(kc == 0),
                            stop=(kc == KC - 1),
                        )
                    alt_copy(copy_flip, vsb[:, tm, :], ps2)
                    copy_flip += 1

            # ---- attention phase, per 128-query tile ----
            for i in range(NT):
                mask_t = maskp.tile([P, T], bf16, tag="mask")
                nc.sync.dma_start(out=mask_t, in_=mask_d.ap()[ts(i, P), :])
                ps_pv = ps_op.tile([P, N], f32, tag="pso")
                attn_sb = attnp.tile([P, N], bf16)
                for h in range(H):
                    probs = probsp.tile([P, T], bf16)
                    den = smallp.tile([P, 2], f32)
                    for hf in range(2):
                        ps_s = ps_bigp.tile([P, 1024], f32, tag="psb")
                        for j2 in range(2):
                            j = hf * 2 + j2
                            nc.tensor.matmul(
                                ps_s[:, ts(j2, 512)],
                                ident_b,
                                mask_t[:, ts(j, 512)],
                                start=True,
                                stop=False,
                            )
                        for j2 in range(2):
                            j = hf * 2 + j2
                            nc.tensor.matmul(
                                ps_s[:, ts(j2, 512)],
                                qkT[:, h, ts(i, P)],
                                qkT[:, H + h, ts(j, 512)],
                                start=False,
                                stop=True,
                            )
                        nc.scalar.activation(
                            out=probs[:, ts(hf, 1024)],
                            in_=ps_s,
                            func=FI.Exp,
                            accum_out=den[:, hf : hf + 1],
                        )
                    denom = smallp.tile([P, 1], f32)
                    nc.vector.tensor_tensor(
                        out=denom,
                        in0=den[:, 0:1],
                        in1=den[:, 1:2],
                        op=mybir.AluOpType.add,
                    )
                    recip = smallp.tile([P, 1], f32)
                    nc.vector.reciprocal(out=recip, in_=denom)
                    probsT = ptp.tile([P, NT, P], bf16)
                    for c in range(4):
                        ps_t = ps_tp.tile([P, 512], bf16, tag="pst")
                        for k in range(4):
                            nc.tensor.matmul(
                                ps_t[:, ts(k, P)],
                                probs[:, ts(c * 4 + k, P)],
                                ident_b,
                                start=(k == 0),
                                stop=(k == 3),
                                is_transpose=True,
                            )
                        nc.vector.tensor_copy(
                            out=probsT[:, c * 4 : (c + 1) * 4, :],
                            in_=ps_t.rearrange("p (c q) -> p c q", c=4),
                        )
                        # PV for this chunk right away: PE follows each copy
                        for m in range(c * 4, c * 4 + 4):
                            nc.tensor.matmul(
                                ps_pv[:, ts(h, HD)],
                                probsT[:, m, :],
                                vsb[:, m, ts(h, HD)],
                                start=(m == 0),
                                stop=(m == NT - 1),
                            )
                    nc.vector.tensor_scalar_mul(
                        out=attn_sb[:, ts(h, HD)],
                        in0=ps_pv[:, ts(h, HD)],
                        scalar1=recip,
                    )
                # project: out[tok, feat] = attnT.T @ Wp + obias
                ps_at = ps_tp.tile([P, N], bf16, tag="pst")
                for k in range(KC):
                    nc.tensor.matmul(
                        ps_at[:, ts(k, P)],
                        attn_sb[:, ts(k, P)],
                        ident_b,
                        start=(k == 0),
                        stop=(k == KC - 1),
                        is_transpose=True,
                    )
                attnT = attntp.tile([P, KC, P], bf16)
                alt_copy(
                    copy_flip, attnT, ps_at.rearrange("p (kc q) -> p kc q", kc=KC)
                )
                copy_flip += 1
                ps_pr = ps_op.tile([P, N], f32, tag="pso")
                for c in range(KC):
                    nc.tensor.matmul(
                        ps_pr,
                        attnT[:, c, :],
                        wp_sb[:, c, :],
                        start=(c == 0),
                        stop=(c == KC - 1),
                    )
                out_sb = outp.tile([P, N], f32)
                nc.vector.tensor_tensor(
                    out=out_sb, in0=ps_pr, in1=ob_bc, op=mybir.AluOpType.add
                )
                nc.sync.dma_start(out=out_d.ap()[ts(i, P), :], in_=out_sb)

    nc.compile()
    return nc


def _get_nc():
    if "nc" not in _CACHE:
        _CACHE["nc"] = _build_nc()
    return _CACHE["nc"]


def _prep_host(x, pos_emb, ln_w, ln_b, aff_w, aff_b, W_qkv, mask_table, W_proj):
    f = np.float32
    bf = ml_dtypes.bfloat16
    x = np.asarray(x, f)
    pos_emb = np.asarray(pos_emb)
    ln_w = np.asarray(ln_w, f)
    ln_b = np.asarray(ln_b, f)
    aff_w = np.asarray(aff_w, f)
    aff_b = np.asarray(aff_b, f)
    W_qkv = np.asarray(W_qkv, f)
    mask_table = np.asarray(mask_table)
    W_proj = np.asarray(W_proj, f)

    s = ln_w * aff_w
    c = ln_b * aff_w + aff_b
    Wf = (s[:, None] * W_qkv).astype(f)
    bf_ = (c @ W_qkv).astype(f)
    scale = f(1.0 / np.sqrt(HD))
    Wqk = np.concatenate([Wf[:, :N] * scale, Wf[:, N : 2 * N]], axis=1)
    Wqk = np.ascontiguousarray(Wqk).astype(bf)
    bqk = np.concatenate([bf_[:N] * scale, bf_[N : 2 * N]]).astype(f)
    Wv = np.ascontiguousarray(Wf[:, 2 * N :]).astype(bf)
    bv = bf_[2 * N :]
    obias = (bv @ W_proj).astype(f)
    Wp = np.ascontiguousarray(W_proj).astype(bf)
    maskadd = np.where(mask_table[pos_emb], f(0.0), f(NEG)).astype(bf)
    return x, maskadd, Wqk, Wv, Wp, bqk, obias


def _install_ntff_hook():
    """Provide the antenv.axon_hooks shim missing from this image so
    run_bass_kernel_spmd(trace=True) can capture NTFF profiles."""
    import types

    try:
        from antenv.axon_hooks import get_axon_ntff_profile_hook  # noqa: F401

        return
    except ImportError:
        pass
    try:
        import antenv
        from trn_agent_boot.trn_boot import _ntff_profile_via_ctypes

        hook = _ntff_profile_via_ctypes("/opt/axon/libaxon_pjrt.so")
        mod = types.ModuleType("antenv.axon_hooks")
        _h = [hook]
        mod.set_axon_ntff_profile_hook = lambda h: _h.__setitem__(0, h)
        mod.get_axon_ntff_profile_hook = lambda: _h[0]
        sys.modules["antenv.axon_hooks"] = mod
        antenv.axon_hooks = mod
    except Exception as e:  # pragma: no cover
        print(f"ntff hook install failed: {e}")


def kernel(x, pos_emb, ln_w, ln_b, aff_w, aff_b, W_qkv, mask_table, W_proj):
    global LAST_RESULTS
    from concourse.bass_utils import run_bass_kernel_spmd

    x, maskadd, Wqk, Wv, Wp, bqk, obias = _prep_host(
        x, pos_emb, ln_w, ln_b, aff_w, aff_b, W_qkv, mask_table, W_proj
    )
    nc = _get_nc()
    in_maps = [
        {
            "x": np.ascontiguousarray(x[i]),
            "maskadd": maskadd,
            "wqk": Wqk,
            "wv": Wv,
            "wproj": Wp,
            "bqk": bqk,
            "obias": obias,
        }
        for i in range(B)
    ]
    trace = bool(int(os.environ.get("ATTN_TRACE", "0")))
    if trace:
        _install_ntff_hook()
    res = run_bass_kernel_spmd(
        nc, in_maps, core_ids=list(range(B)), trace=trace
    )
    LAST_RESULTS = res
    out = np.stack([np.asarray(r["out"]) for r in res.results], axis=0)
    return out.astype(np.float32)


# revision 29
# speedup vs baseline: 1.0163x; 1.0055x over previous
"""Trainium2 Bass kernel for nn_AttnHead_81028853006993.

LayerNorm + affine + fused QKV + 4-head attention with gathered relative-position
mask + output projection, for x:[8, 2048, 512] f32.

Sharding: data-parallel over batch — 8 batches onto 8 NeuronCores, no collectives.
Each core runs the full per-batch attention head stack. All matmuls in bf16
(f32 matmuls cost two PE passes on TRN2); statistics/softmax accumulation in f32.
"""

import os
import sys

import numpy as np

for _p in ("/opt/trn_rl_repo",):
    if _p not in sys.path:
        sys.path.insert(0, _p)

import ml_dtypes  # noqa: E402

B, T, N = 8, 2048, 512
H, HD = 4, 128
P = 128
NT = T // P  # 16 token tiles
KC = N // P  # 4 embed chunks
FQK = 2 * N // P  # 8 feature chunks for fused QK
EPS = 1e-5
NEG = -1e9

LAST_RESULTS = None
_CACHE = {}


def _build_nc():
    import concourse.bacc as bacc
    import concourse.mybir as mybir
    import concourse.tile as tile
    from concourse.bass import AP, ts
    from concourse.masks import make_identity

    f32 = mybir.dt.float32
    bf16 = mybir.dt.bfloat16
    FI = mybir.ActivationFunctionType

    nc = bacc.Bacc("TRN2", target_bir_lowering=False, debug=False, num_devices=8)

    x_d = nc.dram_tensor("x", [T, N], f32, kind="ExternalInput")
    mask_d = nc.dram_tensor("maskadd", [T, T], bf16, kind="ExternalInput")
    wqk_d = nc.dram_tensor("wqk", [N, 2 * N], bf16, kind="ExternalInput")
    wv_d = nc.dram_tensor("wv", [N, N], bf16, kind="ExternalInput")
    wp_d = nc.dram_tensor("wproj", [N, N], bf16, kind="ExternalInput")
    bqk_d = nc.dram_tensor("bqk", [2 * N], f32, kind="ExternalInput")
    ob_d = nc.dram_tensor("obias", [N], f32, kind="ExternalInput")
    out_d = nc.dram_tensor("out", [T, N], f32, kind="ExternalOutput")

    with tile.TileContext(nc) as tc:
        from contextlib import ExitStack

        with ExitStack() as ctx:
            singles = ctx.enter_context(tc.tile_pool(name="singles", bufs=1))
            big = ctx.enter_context(tc.tile_pool(name="big", bufs=1))
            xtp = ctx.enter_context(tc.tile_pool(name="xtp", bufs=2))
            lnx = ctx.enter_context(tc.tile_pool(name="lnx", bufs=3))
            smallp = ctx.enter_context(tc.tile_pool(name="smallp", bufs=8))
            maskp = ctx.enter_context(tc.tile_pool(name="maskp", bufs=3))
            probsp = ctx.enter_context(tc.tile_pool(name="probsp", bufs=3))
            ptp = ctx.enter_context(tc.tile_pool(name="ptp", bufs=3))
            attnp = ctx.enter_context(tc.tile_pool(name="attnp", bufs=2))
            attntp = ctx.enter_context(tc.tile_pool(name="attntp", bufs=2))
            outp = ctx.enter_context(tc.tile_pool(name="outp", bufs=3))
            # PSUM: 2x [128,1024]f32 (4) + 2x [128,512] (2) + 2x [128,512] (2)
            ps_bigp = ctx.enter_context(
                tc.tile_pool(name="ps_bigp", bufs=2, space="PSUM")
            )
            ps_tp = ctx.enter_context(tc.tile_pool(name="ps_tp", bufs=2, space="PSUM"))
            ps_op = ctx.enter_context(tc.tile_pool(name="ps_op", bufs=2, space="PSUM"))

            # ---- constants / weights ----
            ident_b = singles.tile([P, P], bf16)
            make_identity(nc, ident_b)
            eps_t = singles.tile([P, 1], f32)
            nc.vector.memset(eps_t, EPS)

            wqk_sb = singles.tile([P, KC, 2 * N], bf16)
            nc.sync.dma_start(
                out=wqk_sb, in_=wqk_d.ap().rearrange("(kc p) f -> p kc f", p=P)
            )
            wv_sb = singles.tile([P, KC, N], bf16)
            nc.sync.dma_start(
                out=wv_sb, in_=wv_d.ap().rearrange("(kc p) f -> p kc f", p=P)
            )
            wp_sb = singles.tile([P, KC, N], bf16)
            nc.sync.dma_start(
                out=wp_sb, in_=wp_d.ap().rearrange("(kc p) f -> p kc f", p=P)
            )
            bqk_sb = singles.tile([P, FQK], f32)
            nc.sync.dma_start(
                out=bqk_sb, in_=bqk_d.ap().rearrange("(fc p) -> p fc", p=P)
            )
            # obias broadcast to all 128 partitions (DMA partition-stride 0)
            ob_bc = singles.tile([P, N], f32)
            _o = ob_d.ap()
            nc.sync.dma_start(
                out=ob_bc,
                in_=AP(tensor=_o.tensor, offset=_o.offset, ap=[[0, P]] + list(_o.ap)),
            )

            qkT = big.tile([P, FQK, T], bf16)  # Q^T,K^T feature-major
            vsb = big.tile([P, NT, N], bf16)  # V token-major

            def alt_copy(idx, out, in_):
                if idx % 2 == 0:
                    nc.vector.tensor_copy(out=out, in_=in_)
                else:
                    nc.scalar.copy(out=out, in_=in_)

            # ---- fused LN + QKV phase, per 512-token chunk ----
            copy_flip = 0
            for tj in range(4):
                xtc = xtp.tile([P, KC, 4 * P], bf16)  # x-hat^T for this token chunk
                for s in range(4):
                    i = tj * 4 + s
                    x_tile = lnx.tile([P, N], f32)
                    nc.sync.dma_start(out=x_tile, in_=x_d.ap()[ts(i, P), :])
                    stats = smallp.tile([P, 6], f32)
                    nc.vector.bn_stats(out=stats, in_=x_tile)
                    mv = smallp.tile([P, 2], f32)
                    nc.vector.bn_aggr(out=mv, in_=stats)
                    sig = smallp.tile([P, 1], f32)
                    nc.scalar.activation(
                        out=sig, in_=mv[:, 1:2], func=FI.Sqrt, bias=eps_t
                    )
                    rstd = smallp.tile([P, 1], f32)
                    nc.vector.reciprocal(out=rstd, in_=sig)
                    # x-hat = (x - mean) * rstd, cast to bf16
                    xh = lnx.tile([P, N], bf16)
                    nc.vector.tensor_scalar(
                        out=xh,
                        in0=x_tile,
                        scalar1=mv[:, 0:1],
                        scalar2=rstd,
                        op0=mybir.AluOpType.subtract,
                        op1=mybir.AluOpType.mult,
                    )
                    ps_x = ps_tp.tile([P, 4 * P], bf16, tag="pst")
                    for kc in range(KC):
                        nc.tensor.matmul(
                            ps_x[:, ts(kc, P)],
                            xh[:, ts(kc, P)],
                            ident_b,
                            start=(kc == 0),
                            stop=(kc == KC - 1),
                            is_transpose=True,
                        )
                    alt_copy(
                        copy_flip,
                        xtc[:, :, ts(s, P)],
                        ps_x.rearrange("p (kc q) -> p kc q", kc=KC),
                    )
                    copy_flip += 1
                # QK^T for this token chunk: out[feat, tok]
                for g in range(4):
                    ps = ps_bigp.tile([P, 1024], f32, tag="psb")
                    for half in range(2):
                        fc = g * 2 + half
                        for kc in range(KC):
                            nc.tensor.matmul(
                                ps[:, ts(half, 512)],
                                wqk_sb[:, kc, ts(fc, P)],
                                xtc[:, kc, :],
                                start=(kc == 0),
                                stop=(kc == KC - 1),
                            )
                    for half in range(2):
                        fc = g * 2 + half
                        if copy_flip % 2 == 0:
                            nc.vector.tensor_scalar_add(
                                out=qkT[:, fc, ts(tj, 512)],
                                in0=ps[:, ts(half, 512)],
                                scalar1=bqk_sb[:, fc : fc + 1],
                            )
                        else:
                            nc.scalar.activation(
                                out=qkT[:, fc, ts(tj, 512)],
                                in_=ps[:, ts(half, 512)],
                                func=FI.Identity,
                                bias=bqk_sb[:, fc : fc + 1],
                            )
                        copy_flip += 1
                # V for this token chunk: out[tok, feat] -> bf16
                for s in range(4):
                    tm = tj * 4 + s
                    ps2 = ps_op.tile([P, N], f32, tag="pso")
                    for kc in range(KC):
                        nc.tensor.matmul(
                            ps2,
                            xtc[:, kc, ts(s, P)],
                            wv_sb[:, kc, :],
                            start=(kc == 0),
                            stop=(kc == KC - 1),
                        )
                    alt_copy(copy_flip, vsb[:, tm, :], ps2)
                    copy_flip += 1

            # ---- attention phase, per 128-query tile ----
            for i in range(NT):
                mask_t = maskp.tile([P, T], bf16, tag="mask")
                nc.sync.dma_start(out=mask_t, in_=mask_d.ap()[ts(i, P), :])
                ps_pv = ps_op.tile([P, N], f32, tag="pso")
                attn_sb = attnp.tile([P, N], bf16)
                for h in range(H):
                    probs = probsp.tile([P, T], bf16)
                    den = smallp.tile([P, 2], f32)
                    for hf in range(2):
                        ps_s = ps_bigp.tile([P, 1024], f32, tag="psb")
                        for j2 in range(2):
                            j = hf * 2 + j2
                            nc.tensor.matmul(
                                ps_s[:, ts(j2, 512)],
                                ident_b,
                                mask_t[:, ts(j, 512)],
                                start=True,
                                stop=False,
                            )
                        for j2 in range(2):
                            j = hf * 2 + j2
                            nc.tensor.matmul(
                                ps_s[:, ts(j2, 512)],
                                qkT[:, h, ts(i, P)],
                                qkT[:, H + h, ts(j, 512)],
                                start=False,
                                stop=True,
                            )
                        nc.scalar.activation(
                            out=probs[:, ts(hf, 1024)],
                            in_=ps_s,
                            func=FI.Exp,
                            accum_out=den[:, hf : hf + 1],
                        )
                    denom = smallp.tile([P, 1], f32)
                    nc.vector.tensor_tensor(
                        out=denom,
                        in0=den[:, 0:1],
                        in1=den[:, 1:2],
                        op=mybir.AluOpType.add,
                    )
                    recip = smallp.tile([P, 1], f32)
                    nc.vector.reciprocal(out=recip, in_=denom)
                    probsT = ptp.tile([P, NT, P], bf16)
                    for c in range(4):
                        ps_t = ps_tp.tile([P, 512], bf16, tag="pst")
                        for k in range(4):
                            nc.tensor.matmul(
                                ps_t[:, ts(k, P)],
                                probs[:, ts(c * 4 + k, P)],
                                ident_b,
                                start=(k == 0),
                                stop=(k == 3),
                                is_transpose=True,
                            )
                        nc.vector.tensor_copy(
                            out=probsT[:, c * 4 : (c + 1) * 4, :],
                            in_=ps_t.rearrange("p (c q) -> p c q", c=4),
                        )
                    for m in range(NT):
                        nc.tensor.matmul(
                            ps_pv[:, ts(h, HD)],
                            probsT[:, m, :],
                            vsb[:, m, ts(h, HD)],
                            start=(m == 0),
                            stop=(m == NT - 1),
                        )
                    nc.vector.tensor_scalar_mul(
                        out=attn_sb[:, ts(h, HD)],
                        in0=ps_pv[:, ts(h, HD)],
                        scalar1=recip,
                    )
                # project: out[tok, feat] = attnT.T @ Wp + obias
                ps_at = ps_tp.tile([P, N], bf16, tag="pst")
                for k in range(KC):
                    nc.tensor.matmul(
                        ps_at[:, ts(k, P)],
                        attn_sb[:, ts(k, P)],
                        ident_b,
                        start=(k == 0),
                        stop=(k == KC - 1),
                        is_transpose=True,
                    )
                attnT = attntp.tile([P, KC, P], bf16)
                alt_copy(
                    copy_flip, attnT, ps_at.rearrange("p (kc q) -> p kc q", kc=KC)
                )
                copy_flip += 1
                ps_pr = ps_op.tile([P, N], f32, tag="pso")
                for c in range(KC):
                    nc.tensor.matmul(
                        ps_pr,
                        attnT[:, c, :],
                        wp_sb[:, c, :],
                        start=(c == 0),
                        stop=(c == KC - 1),
                    )
                out_sb = outp.tile([P, N], f32)
                nc.vector.tensor_tensor(
                    out=out_sb, in0=ps_pr, in1=ob_bc, op=mybir.AluOpType.add
                )
                nc.sync.dma_start(out=out_d.ap()[ts(i, P), :], in_=out_sb)

    nc.compile()
    return nc


def _get_nc():
    if "nc" not in _CACHE:
        _CACHE["nc"] = _build_nc()
    return _CACHE["nc"]


def _prep_host(x, pos_emb, ln_w, ln_b, aff_w, aff_b, W_qkv, mask_table, W_proj):
    f = np.float32
    bf = ml_dtypes.bfloat16
    x = np.asarray(x, f)
    pos_emb = np.asarray(pos_emb)
    ln_w = np.asarray(ln_w, f)
    ln_b = np.asarray(ln_b, f)
    aff_w = np.asarray(aff_w, f)
    aff_b = np.asarray(aff_b, f)
    W_qkv = np.asarray(W_qkv, f)
    mask_table = np.asarray(mask_table)
    W_proj = np.asarray(W_proj, f)

    s = ln_w * aff_w
    c = ln_b * aff_w + aff_b
    Wf = (s[:, None] * W_qkv).astype(f)
    bf_ = (c @ W_qkv).astype(f)
    scale = f(1.0 / np.sqrt(HD))
    Wqk = np.concatenate([Wf[:, :N] * scale, Wf[:, N : 2 * N]], axis=1)
    Wqk = np.ascontiguousarray(Wqk).astype(bf)
    bqk = np.concatenate([bf_[:N] * scale, bf_[N : 2 * N]]).astype(f)
    Wv = np.ascontiguousarray(Wf[:, 2 * N :]).astype(bf)
    bv = bf_[2 * N :]
    obias = (bv @ W_proj).astype(f)
    Wp = np.ascontiguousarray(W_proj).astype(bf)
    maskadd = np.where(mask_table[pos_emb], f(0.0), f(NEG)).astype(bf)
    return x, maskadd, Wqk, Wv, Wp, bqk, obias


def _install_ntff_hook():
    """Provide the antenv.axon_hooks shim missing from this image so
    run_bass_kernel_spmd(trace=True) can capture NTFF profiles."""
    import types

    try:
        from antenv.axon_hooks import get_axon_ntff_profile_hook  # noqa: F401

        return
    except ImportError:
        pass
    try:
        import antenv
        from trn_agent_boot.trn_boot import _ntff_profile_via_ctypes

        hook = _ntff_profile_via_ctypes("/opt/axon/libaxon_pjrt.so")
        mod = types.ModuleType("antenv.axon_hooks")
        _h = [hook]
        mod.set_axon_ntff_profile_hook = lambda h: _h.__setitem__(0, h)
        mod.get_axon_ntff_profile_hook = lambda: _h[0]
        sys.modules["antenv.axon_hooks"] = mod
        antenv.axon_hooks = mod
    except Exception as e:  # pragma: no cover
        print(f"ntff hook install failed: {e}")


def kernel(x, pos_emb, ln_w, ln_b, aff_w, aff_b, W_qkv, mask_table, W_proj):
    global LAST_RESULTS
    from concourse.bass_utils import run_bass_kernel_spmd

    x, maskadd, Wqk, Wv, Wp, bqk, obias = _prep_host(
        x, pos_emb, ln_w, ln_b, aff_w, aff_b, W_qkv, mask_table, W_proj
    )
    nc = _get_nc()
    in_maps = [
        {
            "x": np.ascontiguousarray(x[i]),
            "maskadd": maskadd,
            "wqk": Wqk,
            "wv": Wv,
            "wproj": Wp,
            "bqk": bqk,
            "obias": obias,
        }
        for i in range(B)
    ]
    trace = bool(int(os.environ.get("ATTN_TRACE", "0")))
    if trace:
        _install_ntff_hook()
    res = run_bass_kernel_spmd(
        nc, in_maps, core_ids=list(range(B)), trace=trace
    )
    LAST_RESULTS = res
    out = np.stack([np.asarray(r["out"]) for r in res.results], axis=0)
    return out.astype(np.float32)


# revision 30
# speedup vs baseline: 1.0209x; 1.0045x over previous
"""Trainium2 Bass kernel for nn_AttnHead_81028853006993.

LayerNorm + affine + fused QKV + 4-head attention with gathered relative-position
mask + output projection, for x:[8, 2048, 512] f32.

Sharding: data-parallel over batch — 8 batches onto 8 NeuronCores, no collectives.
Each core runs the full per-batch attention head stack. All matmuls in bf16
(f32 matmuls cost two PE passes on TRN2); statistics/softmax accumulation in f32.
"""

import os
import sys

import numpy as np

for _p in ("/opt/trn_rl_repo",):
    if _p not in sys.path:
        sys.path.insert(0, _p)

import ml_dtypes  # noqa: E402

B, T, N = 8, 2048, 512
H, HD = 4, 128
P = 128
NT = T // P  # 16 token tiles
KC = N // P  # 4 embed chunks
FQK = 2 * N // P  # 8 feature chunks for fused QK
EPS = 1e-5
NEG = -1e9

LAST_RESULTS = None
_CACHE = {}


def _build_nc():
    import concourse.bacc as bacc
    import concourse.mybir as mybir
    import concourse.tile as tile
    from concourse.bass import AP, ts
    from concourse.masks import make_identity

    f32 = mybir.dt.float32
    bf16 = mybir.dt.bfloat16
    FI = mybir.ActivationFunctionType

    nc = bacc.Bacc("TRN2", target_bir_lowering=False, debug=False, num_devices=8)

    x_d = nc.dram_tensor("x", [T, N], f32, kind="ExternalInput")
    mask_d = nc.dram_tensor("maskadd", [T, T], bf16, kind="ExternalInput")
    wqk_d = nc.dram_tensor("wqk", [N, 2 * N], bf16, kind="ExternalInput")
    wv_d = nc.dram_tensor("wv", [N, N], bf16, kind="ExternalInput")
    wp_d = nc.dram_tensor("wproj", [N, N], bf16, kind="ExternalInput")
    bqk_d = nc.dram_tensor("bqk", [2 * N], f32, kind="ExternalInput")
    ob_d = nc.dram_tensor("obias", [N], f32, kind="ExternalInput")
    out_d = nc.dram_tensor("out", [T, N], f32, kind="ExternalOutput")

    with tile.TileContext(nc) as tc:
        from contextlib import ExitStack

        with ExitStack() as ctx:
            singles = ctx.enter_context(tc.tile_pool(name="singles", bufs=1))
            big = ctx.enter_context(tc.tile_pool(name="big", bufs=1))
            xtp = ctx.enter_context(tc.tile_pool(name="xtp", bufs=2))
            lnx = ctx.enter_context(tc.tile_pool(name="lnx", bufs=3))
            smallp = ctx.enter_context(tc.tile_pool(name="smallp", bufs=8))
            maskp = ctx.enter_context(tc.tile_pool(name="maskp", bufs=4))
            probsp = ctx.enter_context(tc.tile_pool(name="probsp", bufs=4))
            ptp = ctx.enter_context(tc.tile_pool(name="ptp", bufs=4))
            attnp = ctx.enter_context(tc.tile_pool(name="attnp", bufs=2))
            attntp = ctx.enter_context(tc.tile_pool(name="attntp", bufs=2))
            outp = ctx.enter_context(tc.tile_pool(name="outp", bufs=4))
            # PSUM: 2x [128,1024]f32 (4) + 2x [128,512] (2) + 2x [128,512] (2)
            ps_bigp = ctx.enter_context(
                tc.tile_pool(name="ps_bigp", bufs=2, space="PSUM")
            )
            ps_tp = ctx.enter_context(tc.tile_pool(name="ps_tp", bufs=2, space="PSUM"))
            ps_op = ctx.enter_context(tc.tile_pool(name="ps_op", bufs=2, space="PSUM"))

            # ---- constants / weights ----
            ident_b = singles.tile([P, P], bf16)
            make_identity(nc, ident_b)
            eps_t = singles.tile([P, 1], f32)
            nc.vector.memset(eps_t, EPS)

            wqk_sb = singles.tile([P, KC, 2 * N], bf16)
            nc.sync.dma_start(
                out=wqk_sb, in_=wqk_d.ap().rearrange("(kc p) f -> p kc f", p=P)
            )
            wv_sb = singles.tile([P, KC, N], bf16)
            nc.sync.dma_start(
                out=wv_sb, in_=wv_d.ap().rearrange("(kc p) f -> p kc f", p=P)
            )
            wp_sb = singles.tile([P, KC, N], bf16)
            nc.sync.dma_start(
                out=wp_sb, in_=wp_d.ap().rearrange("(kc p) f -> p kc f", p=P)
            )
            bqk_sb = singles.tile([P, FQK], f32)
            nc.sync.dma_start(
                out=bqk_sb, in_=bqk_d.ap().rearrange("(fc p) -> p fc", p=P)
            )
            # obias broadcast to all 128 partitions (DMA partition-stride 0)
            ob_bc = singles.tile([P, N], f32)
            _o = ob_d.ap()
            nc.sync.dma_start(
                out=ob_bc,
                in_=AP(tensor=_o.tensor, offset=_o.offset, ap=[[0, P]] + list(_o.ap)),
            )

            qkT = big.tile([P, FQK, T], bf16)  # Q^T,K^T feature-major
            vsb = big.tile([P, NT, N], bf16)  # V token-major

            def alt_copy(idx, out, in_):
                if idx % 2 == 0:
                    nc.vector.tensor_copy(out=out, in_=in_)
                else:
                    nc.scalar.copy(out=out, in_=in_)

            # ---- fused LN + QKV phase, per 512-token chunk ----
            copy_flip = 0
            for tj in range(4):
                xtc = xtp.tile([P, KC, 4 * P], bf16)  # x-hat^T for this token chunk
                for s in range(4):
                    i = tj * 4 + s
                    x_tile = lnx.tile([P, N], f32)
                    nc.sync.dma_start(out=x_tile, in_=x_d.ap()[ts(i, P), :])
                    stats = smallp.tile([P, 6], f32)
                    nc.vector.bn_stats(out=stats, in_=x_tile)
                    mv = smallp.tile([P, 2], f32)
                    nc.vector.bn_aggr(out=mv, in_=stats)
                    sig = smallp.tile([P, 1], f32)
                    nc.scalar.activation(
                        out=sig, in_=mv[:, 1:2], func=FI.Sqrt, bias=eps_t
                    )
                    rstd = smallp.tile([P, 1], f32)
                    nc.vector.reciprocal(out=rstd, in_=sig)
                    # x-hat = (x - mean) * rstd, cast to bf16
                    xh = lnx.tile([P, N], bf16)
                    nc.vector.tensor_scalar(
                        out=xh,
                        in0=x_tile,
                        scalar1=mv[:, 0:1],
                        scalar2=rstd,
                        op0=mybir.AluOpType.subtract,
                        op1=mybir.AluOpType.mult,
                    )
                    ps_x = ps_tp.tile([P, 4 * P], bf16, tag="pst")
                    for kc in range(KC):
                        nc.tensor.matmul(
                            ps_x[:, ts(kc, P)],
                            xh[:, ts(kc, P)],
                            ident_b,
                            start=(kc == 0),
                            stop=(kc == KC - 1),
                            is_transpose=True,
                        )
                    alt_copy(
                        copy_flip,
                        xtc[:, :, ts(s, P)],
                        ps_x.rearrange("p (kc q) -> p kc q", kc=KC),
                    )
                    copy_flip += 1
                # QK^T for this token chunk: out[feat, tok]
                for g in range(4):
                    ps = ps_bigp.tile([P, 1024], f32, tag="psb")
                    for half in range(2):
                        fc = g * 2 + half
                        for kc in range(KC):
                            nc.tensor.matmul(
                                ps[:, ts(half, 512)],
                                wqk_sb[:, kc, ts(fc, P)],
                                xtc[:, kc, :],
                                start=(kc == 0),
                                stop=(kc == KC - 1),
                            )
                    for half in range(2):
                        fc = g * 2 + half
                        if copy_flip % 2 == 0:
                            nc.vector.tensor_scalar_add(
                                out=qkT[:, fc, ts(tj, 512)],
                                in0=ps[:, ts(half, 512)],
                                scalar1=bqk_sb[:, fc : fc + 1],
                            )
                        else:
                            nc.scalar.activation(
                                out=qkT[:, fc, ts(tj, 512)],
                                in_=ps[:, ts(half, 512)],
                                func=FI.Identity,
                                bias=bqk_sb[:, fc : fc + 1],
                            )
                        copy_flip += 1
                # V for this token chunk: out[tok, feat] -> bf16
                for s in range(4):
                    tm = tj * 4 + s
                    ps2 = ps_op.tile([P, N], f32, tag="pso")
                    for kc in range(KC):
                        nc.tensor.matmul(
                            ps2,
                            xtc[:, kc, ts(s, P)],
                            wv_sb[:, kc, :],
                            start=(kc == 0),
                            stop=(kc == KC - 1),
                        )
                    alt_copy(copy_flip, vsb[:, tm, :], ps2)
                    copy_flip += 1

            # ---- attention phase, per 128-query tile ----
            for i in range(NT):
                mask_t = maskp.tile([P, T], bf16, tag="mask")
                nc.sync.dma_start(out=mask_t, in_=mask_d.ap()[ts(i, P), :])
                ps_pv = ps_op.tile([P, N], f32, tag="pso")
                attn_sb = attnp.tile([P, N], bf16)
                for h in range(H):
                    probs = probsp.tile([P, T], bf16)
                    den = smallp.tile([P, 2], f32)
                    for hf in range(2):
                        ps_s = ps_bigp.tile([P, 1024], f32, tag="psb")
                        for j2 in range(2):
                            j = hf * 2 + j2
                            nc.tensor.matmul(
                                ps_s[:, ts(j2, 512)],
                                ident_b,
                                mask_t[:, ts(j, 512)],
                                start=True,
                                stop=False,
                            )
                        for j2 in range(2):
                            j = hf * 2 + j2
                            nc.tensor.matmul(
                                ps_s[:, ts(j2, 512)],
                                qkT[:, h, ts(i, P)],
                                qkT[:, H + h, ts(j, 512)],
                                start=False,
                                stop=True,
                            )
                        nc.scalar.activation(
                            out=probs[:, ts(hf, 1024)],
                            in_=ps_s,
                            func=FI.Exp,
                            accum_out=den[:, hf : hf + 1],
                        )
                    denom = smallp.tile([P, 1], f32)
                    nc.vector.tensor_tensor(
                        out=denom,
                        in0=den[:, 0:1],
                        in1=den[:, 1:2],
                        op=mybir.AluOpType.add,
                    )
                    recip = smallp.tile([P, 1], f32)
                    nc.vector.reciprocal(out=recip, in_=denom)
                    probsT = ptp.tile([P, NT, P], bf16)
                    for c in range(4):
                        ps_t = ps_tp.tile([P, 512], bf16, tag="pst")
                        for k in range(4):
                            nc.tensor.matmul(
                                ps_t[:, ts(k, P)],
                                probs[:, ts(c * 4 + k, P)],
                                ident_b,
                                start=(k == 0),
                                stop=(k == 3),
                                is_transpose=True,
                            )
                        nc.vector.tensor_copy(
                            out=probsT[:, c * 4 : (c + 1) * 4, :],
                            in_=ps_t.rearrange("p (c q) -> p c q", c=4),
                        )
                    for m in range(NT):
                        nc.tensor.matmul(
                            ps_pv[:, ts(h, HD)],
                            probsT[:, m, :],
                            vsb[:, m, ts(h, HD)],
                            start=(m == 0),
                            stop=(m == NT - 1),
                        )
                    nc.vector.tensor_scalar_mul(
                        out=attn_sb[:, ts(h, HD)],
                        in0=ps_pv[:, ts(h, HD)],
                        scalar1=recip,
                    )
                # project: out[tok, feat] = attnT.T @ Wp + obias
                ps_at = ps_tp.tile([P, N], bf16, tag="pst")
                for k in range(KC):
                    nc.tensor.matmul(
                        ps_at[:, ts(k, P)],
                        attn_sb[:, ts(k, P)],
                        ident_b,
                        start=(k == 0),
                        stop=(k == KC - 1),
                        is_transpose=True,
                    )
                attnT = attntp.tile([P, KC, P], bf16)
                alt_copy(
                    copy_flip, attnT, ps_at.rearrange("p (kc q) -> p kc q", kc=KC)
                )
                copy_flip += 1
                ps_pr = ps_op.tile([P, N], f32, tag="pso")
                for c in range(KC):
                    nc.tensor.matmul(
                        ps_pr,
                        attnT[:, c, :],
                        wp_sb[:, c, :],
                        start=(c == 0),
                        stop=(c == KC - 1),
                    )
                out_sb = outp.tile([P, N], f32)
                nc.vector.tensor_tensor(
                    out=out_sb, in0=ps_pr, in1=ob_bc, op=mybir.AluOpType.add
                )
                nc.sync.dma_start(out=out_d.ap()[ts(i, P), :], in_=out_sb)

    nc.compile()
    return nc


def _get_nc():
    if "nc" not in _CACHE:
        _CACHE["nc"] = _build_nc()
    return _CACHE["nc"]


def _prep_host(x, pos_emb, ln_w, ln_b, aff_w, aff_b, W_qkv, mask_table, W_proj):
    f = np.float32
    bf = ml_dtypes.bfloat16
    x = np.asarray(x, f)
    pos_emb = np.asarray(pos_emb)
    ln_w = np.asarray(ln_w, f)
    ln_b = np.asarray(ln_b, f)
    aff_w = np.asarray(aff_w, f)
    aff_b = np.asarray(aff_b, f)
    W_qkv = np.asarray(W_qkv, f)
    mask_table = np.asarray(mask_table)
    W_proj = np.asarray(W_proj, f)

    s = ln_w * aff_w
    c = ln_b * aff_w + aff_b
    Wf = (s[:, None] * W_qkv).astype(f)
    bf_ = (c @ W_qkv).astype(f)
    scale = f(1.0 / np.sqrt(HD))
    Wqk = np.concatenate([Wf[:, :N] * scale, Wf[:, N : 2 * N]], axis=1)
    Wqk = np.ascontiguousarray(Wqk).astype(bf)
    bqk = np.concatenate([bf_[:N] * scale, bf_[N : 2 * N]]).astype(f)
    Wv = np.ascontiguousarray(Wf[:, 2 * N :]).astype(bf)
    bv = bf_[2 * N :]
    obias = (bv @ W_proj).astype(f)
    Wp = np.ascontiguousarray(W_proj).astype(bf)
    maskadd = np.where(mask_table[pos_emb], f(0.0), f(NEG)).astype(bf)
    return x, maskadd, Wqk, Wv, Wp, bqk, obias


def _install_ntff_hook():
    """Provide the antenv.axon_hooks shim missing from this image so
    run_bass_kernel_spmd(trace=True) can capture NTFF profiles."""
    import types

    try:
        from antenv.axon_hooks import get_axon_ntff_profile_hook  # noqa: F401

        return
    except ImportError:
        pass
    try:
        import antenv
        from trn_agent_boot.trn_boot import _ntff_profile_via_ctypes

        hook = _ntff_profile_via_ctypes("/opt/axon/libaxon_pjrt.so")
        mod = types.ModuleType("antenv.axon_hooks")
        _h = [hook]
        mod.set_axon_ntff_profile_hook = lambda h: _h.__setitem__(0, h)
        mod.get_axon_ntff_profile_hook = lambda: _h[0]
        sys.modules["antenv.axon_hooks"] = mod
        antenv.axon_hooks = mod
    except Exception as e:  # pragma: no cover
        print(f"ntff hook install failed: {e}")


def kernel(x, pos_emb, ln_w, ln_b, aff_w, aff_b, W_qkv, mask_table, W_proj):
    global LAST_RESULTS
    from concourse.bass_utils import run_bass_kernel_spmd

    x, maskadd, Wqk, Wv, Wp, bqk, obias = _prep_host(
        x, pos_emb, ln_w, ln_b, aff_w, aff_b, W_qkv, mask_table, W_proj
    )
    nc = _get_nc()
    in_maps = [
        {
            "x": np.ascontiguousarray(x[i]),
            "maskadd": maskadd,
            "wqk": Wqk,
            "wv": Wv,
            "wproj": Wp,
            "bqk": bqk,
            "obias": obias,
        }
        for i in range(B)
    ]
    trace = bool(int(os.environ.get("ATTN_TRACE", "0")))
    if trace:
        _install_ntff_hook()
    res = run_bass_kernel_spmd(
        nc, in_maps, core_ids=list(range(B)), trace=trace
    )
    LAST_RESULTS = res
    out = np.stack([np.asarray(r["out"]) for r in res.results], axis=0)
    return out.astype(np.float32)
